# revision 21
# baseline (speedup 1.0000x reference)
"""Trainium2 Bass kernel for nn_Attention_91225105367483.

Spatial attention: x[B=2,T=8,H=32,W=32,D=768] -> 16 frames x 1024 tokens.
Data-parallel over frames: 8 cores x 2 frames each. No collectives.

Per-core layout (all hardcoded):
  - host pre-transposes: xT [768, 2048] (d-major), wqkT [768, 1536],
    wvT [768, 768], w_outT [768, 768], RoPE cos/sin packed [128, 1024]
    (2 heads x 64 dims), rotation matrix RT [128,128] implementing
    rotate_half as a matmul.
  - QKV proj: q,k computed TRANSPOSED ([64 hd, seq]); v natural ([seq, 64])
    with a ones column per head so attn@v also emits softmax denominators.
  - scores transposed sT=[keys, queries], 2 heads packed into the PE array
    via tile_position row halves (concurrent); exp on ACT with scale=1/8.
  - softmax skips max-subtraction (scores are O(1), exact to 2ULP).
  - normalize: DVE reciprocal, then 1/r broadcast via a PE ones-matmul into
    the unused rows 64:128 of the attn@v PSUM bank (zero extra PSUM), staged
    to SBUF on ScalarE, one DVE mul per head straight from PSUM.
  - RoPE: q_rot = q*cos + R@(q*sin) on DVE + PE (commuted form keeps the
    sin-mul in cheap bf16 SBUF mode).
  - emission order arranges cross-phase overlap: frame f+1's projections
    and frame f's out-proj fill the PE stalls inside frame f/f+1's
    ACT-bound attention phases.
"""
import sys

sys.path.insert(0, "/opt/trn_rl_repo")

import numpy as np
import ml_dtypes

BF16 = ml_dtypes.bfloat16

B, T, D = 2, 8, 768
NH, HD = 12, 64
NCORES = 8
FPC = 2  # frames per core
NPAIR = NH // 2
NDC = D // 128

_built = {}


def _host_rope(H, W, head_dim):
    """Replicates reference._rope_cos_sin in numpy fp32."""
    half = head_dim // 4
    inv_freq = (1.0 / (10000.0 ** (np.arange(half, dtype=np.float32) / half))).astype(
        np.float32
    )
    th_h = np.arange(H, dtype=np.float32)[:, None] * inv_freq  # [H, half]
    th_w = np.arange(W, dtype=np.float32)[:, None] * inv_freq  # [W, half]
    cos = np.concatenate(
        [
            np.broadcast_to(np.cos(th_h)[:, None, :], (H, W, half)),
            np.broadcast_to(np.cos(th_w)[None, :, :], (H, W, half)),
        ],
        axis=-1,
    )
    sin = np.concatenate(
        [
            np.broadcast_to(np.sin(th_h)[:, None, :], (H, W, half)),
            np.broadcast_to(np.sin(th_w)[None, :, :], (H, W, half)),
        ],
        axis=-1,
    )
    cos = np.repeat(cos, 2, axis=-1).reshape(H * W, head_dim).astype(np.float32)
    sin = np.repeat(sin, 2, axis=-1).reshape(H * W, head_dim).astype(np.float32)
    return cos, sin


def _rot_matT():
    """RT = R.T where (R @ q)[2i] = -q[2i+1], (R @ q)[2i+1] = q[2i]."""
    RT = np.zeros((128, 128), dtype=np.float32)
    for i in range(64):
        RT[2 * i + 1, 2 * i] = -1.0
        RT[2 * i, 2 * i + 1] = 1.0
    return RT


def build_nc(H, W):
    """Builds the per-core Bass program. S = H*W tokens per frame."""
    import concourse.bass as bass
    import concourse.tile as tile
    from concourse import bacc, mybir

    dt = mybir.dt
    S = H * W
    SL = FPC * S
    QCS = 512  # query-chunk size
    NQC = S // QCS
    KT = S // 128  # key tiles per frame
    SC = 512  # token chunk for projections
    NSCF = S // SC  # chunks per frame
    NSTF = S // 128  # s-tiles per frame

    nc = bacc.Bacc("TRN2", target_bir_lowering=False, debug=False)

    xT = nc.dram_tensor("xT", [D, SL], dt.bfloat16, kind="ExternalInput")
    wqkT = nc.dram_tensor("wqkT", [D, 2 * D], dt.bfloat16, kind="ExternalInput")
    wvT = nc.dram_tensor("wvT", [D, D], dt.bfloat16, kind="ExternalInput")
    w_outT = nc.dram_tensor("w_outT", [D, D], dt.bfloat16, kind="ExternalInput")
    cosP = nc.dram_tensor("cosP", [128, S], dt.bfloat16, kind="ExternalInput")
    sinP = nc.dram_tensor("sinP", [128, S], dt.bfloat16, kind="ExternalInput")
    rotT = nc.dram_tensor("rotT", [128, 128], dt.bfloat16, kind="ExternalInput")
    ones64 = nc.dram_tensor("ones64", [1, 64], dt.bfloat16, kind="ExternalInput")
    bias_rep = nc.dram_tensor("bias_rep", [128, D], dt.bfloat16, kind="ExternalInput")
    out = nc.dram_tensor("out", [SL, D], dt.float32, kind="ExternalOutput")

    with tile.TileContext(nc) as tc:
        import contextlib

        ctx = contextlib.ExitStack()
        with ctx:
            const = ctx.enter_context(tc.tile_pool(name="const", bufs=1))
            xt_pool = ctx.enter_context(tc.tile_pool(name="xt", bufs=20))
            qk_pool = ctx.enter_context(tc.tile_pool(name="qk", bufs=1))
            v_pool = ctx.enter_context(tc.tile_pool(name="v", bufs=1))
            ot_pool = ctx.enter_context(tc.tile_pool(name="ot", bufs=1))
            et_pool = ctx.enter_context(tc.tile_pool(name="et", bufs=3))
            rtmp_pool = ctx.enter_context(tc.tile_pool(name="rtmp", bufs=2))
            onorm_pool = ctx.enter_context(tc.tile_pool(name="onorm", bufs=2))
            outsb_pool = ctx.enter_context(tc.tile_pool(name="outsb", bufs=2))
            mm_ps = ctx.enter_context(tc.tile_pool(name="mmps", bufs=2, space="PSUM"))
            sc_ps = ctx.enter_context(tc.tile_pool(name="scps", bufs=2, space="PSUM"))
            av_ps = ctx.enter_context(tc.tile_pool(name="avps", bufs=1, space="PSUM"))

            ActF = mybir.ActivationFunctionType
            scale = 1.0 / np.sqrt(HD)

            # ---- constant DMAs (qk weights first: first compute needs them) ----
            wqk_t = []
            for d in range(NDC):
                t = const.tile([128, 2 * D], dt.bfloat16, tag=f"wqk{d}", name=f"wqk{d}")
                nc.sync.dma_start(t[:], wqkT[d * 128 : (d + 1) * 128, :])
                wqk_t.append(t)
            cos_t = const.tile([128, S], dt.bfloat16, tag="cos")
            nc.sync.dma_start(cos_t[:], cosP[:])
            sin_t = const.tile([128, S], dt.bfloat16, tag="sin")
            nc.sync.dma_start(sin_t[:], sinP[:])
            rot_t = const.tile([128, 128], dt.bfloat16, tag="rot")
            nc.sync.dma_start(rot_t[:], rotT[:])
            ones_t = const.tile([1, 64], dt.bfloat16, tag="ones")
            nc.sync.dma_start(ones_t[:], ones64[:])
            wv_t = [const.tile([128, D], dt.bfloat16, tag=f"wv{d}", name=f"wv{d}")
                    for d in range(NDC)]
            wo_t = [const.tile([128, D], dt.bfloat16, tag=f"wout{d}", name=f"wout{d}")
                    for d in range(NDC)]
            bias_t = const.tile([128, D], dt.bfloat16, tag="bias")

            def dma_late_consts():
                for d in range(NDC):
                    nc.sync.dma_start(wv_t[d][:], wvT[d * 128 : (d + 1) * 128, :])
                for d in range(NDC):
                    nc.sync.dma_start(wo_t[d][:], w_outT[d * 128 : (d + 1) * 128, :])
                nc.sync.dma_start(bias_t[:], bias_rep[:])

            # ---- per-frame tile constructors ----
            def alloc_frame(f):
                qk_q = [qk_pool.tile([128, S], dt.bfloat16, tag=f"q{p}", bufs=2,
                                     name=f"qkq{f}_{p}") for p in range(NPAIR)]
                qk_k = [qk_pool.tile([128, S], dt.bfloat16, tag=f"k{p}", bufs=2,
                                     name=f"qkk{f}_{p}") for p in range(NPAIR)]
                v_sb = [v_pool.tile([128, NH * 65], dt.bfloat16, tag=f"v{i}", bufs=2,
                                    name=f"vsb{f}_{i}") for i in range(NSTF)]
                ot_sb = [ot_pool.tile([128, S], dt.bfloat16, tag=f"ot{d}", bufs=2,
                                      name=f"otsb{f}_{d}") for d in range(NDC)]
                return qk_q, qk_k, v_sb, ot_sb

            def dma_x(f):
                xts = []
                for c in range(NSCF):
                    row = []
                    for d in range(NDC):
                        t = xt_pool.tile([128, SC], dt.bfloat16, tag="xt",
                                         name=f"xt_{f}_{c}_{d}")
                        nc.sync.dma_start(
                            t[:], xT[d * 128 : (d + 1) * 128,
                                     f * S + c * SC : f * S + (c + 1) * SC])
                        row.append(t)
                    xts.append(row)
                return xts

            def emit_qk_pair(f, p, xts, qk_q, qk_k):
                """projection + rope for one head pair."""
                for ti, (tens, et) in enumerate(((qk_q[p], p), (qk_k[p], NPAIR + p))):
                    tens, et = (qk_q[p], p) if ti == 0 else (qk_k[p], NPAIR + p)
                    for c in range(NSCF):
                        ps = mm_ps.tile([128, SC], dt.float32, tag="mm",
                                        name=f"qkps_{f}_{p}_{ti}_{c}")
                        for d in range(NDC):
                            nc.tensor.matmul(
                                ps[:],
                                wqk_t[d][:, et * 128 : (et + 1) * 128],
                                xts[c][d][:],
                                start=(d == 0),
                                stop=(d == NDC - 1),
                            )
                        nc.vector.tensor_copy(tens[:, c * SC : (c + 1) * SC], ps[:])
                # rope in-place on the pair's q/k tiles
                for ti in range(2):
                    tens = qk_q[p] if ti == 0 else qk_k[p]
                    for c in range(S // 512):
                        sl_ = slice(c * 512, (c + 1) * 512)
                        u = rtmp_pool.tile([128, 512], dt.bfloat16, tag="u",
                                           name=f"u_{f}_{p}_{ti}_{c}")
                        nc.vector.tensor_mul(u[:], tens[:, sl_], sin_t[:, sl_])
                        rps = mm_ps.tile([128, 512], dt.float32, tag="mm",
                                         name=f"rps_{f}_{p}_{ti}_{c}")
                        nc.tensor.matmul(rps[:], rot_t[:], u[:], start=True, stop=True)
                        t2 = rtmp_pool.tile([128, 512], dt.bfloat16, tag="t2",
                                            name=f"t2_{f}_{p}_{ti}_{c}")
                        nc.vector.tensor_mul(t2[:], tens[:, sl_], cos_t[:, sl_])
                        nc.vector.tensor_add(tens[:, sl_], rps[:], t2[:])

            def emit_v_stile(f, xts, v_sb, lst):
                """memset + 2 projection chains filling v_sb[lst]."""
                c, st = divmod(lst, SC // 128)
                vv = v_sb[lst][:].rearrange("p (h c) -> p h c", h=NH)
                nc.vector.memset(vv[:, :, 64:65], 1.0)
                for nch in range(2):
                    n0, n1 = (0, 512) if nch == 0 else (512, D)
                    ps = mm_ps.tile([128, 512], dt.float32, tag="mm",
                                    name=f"vps_{f}_{lst}_{nch}")
                    for d in range(NDC):
                        nc.tensor.matmul(
                            ps[:, : n1 - n0],
                            xts[c][d][:, st * 128 : (st + 1) * 128],
                            wv_t[d][:, n0:n1],
                            start=(d == 0),
                            stop=(d == NDC - 1),
                        )
                    h0, h1 = (0, 8) if nch == 0 else (8, NH)
                    pv = ps[:, : n1 - n0].rearrange("p (h c) -> p h c", c=HD)
                    nc.vector.tensor_copy(vv[:, h0:h1, 0:HD], pv[:])

            def emit_attn_unit(f, qc, p, qk_q, qk_k, v_sb, ot_sb, v_inline=None):
                qsl = slice(qc * QCS, (qc + 1) * QCS)
                avp = [
                    av_ps.tile([128, QCS], dt.float32, tag=f"av{hh}", bufs=1,
                               name=f"avp_{f}_{p}_{qc}_{hh}")
                    for hh in range(2)
                ]
                for g in range(KT):
                    if v_inline is not None:
                        v_inline(g)
                    ksl = slice(g * 128, (g + 1) * 128)
                    sp = sc_ps.tile([128, 2 * QCS], dt.float32, tag="sc",
                                    name=f"sp_{f}_{p}_{qc}_{g}")
                    for hh in range(2):
                        rb = 64 * hh
                        nc.tensor.matmul(
                            sp[:, hh * QCS : (hh + 1) * QCS],
                            qk_k[p][rb : rb + 64, ksl],
                            qk_q[p][rb : rb + 64, qsl],
                            start=True,
                            stop=True,
                            tile_position=(rb, 0),
                        )
                    et_t = et_pool.tile([128, 2 * QCS], dt.bfloat16, tag="et",
                                        name=f"et_{f}_{p}_{qc}_{g}")
                    nc.scalar.activation(et_t[:], sp[:], ActF.Exp, scale=float(scale))
                    for hh in range(2):
                        h = 2 * p + hh
                        nc.tensor.matmul(
                            avp[hh][0:65, :],
                            v_sb[g][:, h * 65 : h * 65 + 65],
                            et_t[:, hh * QCS : (hh + 1) * QCS],
                            start=(g == 0),
                            stop=(g == KT - 1),
                        )
                # normalize: evacuate o+r per head with ONE copy (releases the
                # avp bank fast), then 1/r -> PE broadcast -> one DVE mul/head.
                our = [
                    onorm_pool.tile([65, QCS], dt.float32, tag=f"our{hh}",
                                    name=f"our_{f}_{p}_{qc}_{hh}")
                    for hh in range(2)
                ]
                for hh in range(2):
                    nc.vector.tensor_copy(our[hh][:], avp[hh][0:65, :])
                rr2 = onorm_pool.tile([1, 2 * QCS], dt.float32, tag="rr2",
                                      bufs=1, name=f"rr2_{f}_{p}_{qc}")
                nc.vector.tensor_copy(rr2[0:1, 0:QCS], our[0][64:65, :])
                nc.vector.tensor_copy(rr2[0:1, QCS : 2 * QCS], our[1][64:65, :])
                rc2 = onorm_pool.tile([1, 2 * QCS], dt.float32, tag="rc2",
                                      bufs=1, name=f"rc2_{f}_{p}_{qc}")
                nc.vector.reciprocal_approx_fast(rc2[:], rr2[:])
                rcb = onorm_pool.tile([1, 2 * QCS], dt.bfloat16, tag="rcb",
                                      bufs=1, name=f"rcb_{f}_{p}_{qc}")
                nc.vector.tensor_copy(rcb[:], rc2[:])
                bcps = sc_ps.tile([128, QCS], dt.float32, tag="sc",
                                  name=f"bcps_{f}_{p}_{qc}")
                nc.tensor.matmul(bcps[0:64, :], ones_t[:], rcb[0:1, 0:QCS],
                                 start=True, stop=True, tile_position=(0, 0))
                nc.tensor.matmul(bcps[64:128, :], ones_t[:], rcb[0:1, QCS : 2 * QCS],
                                 start=True, stop=True, tile_position=(0, 64))
                nc.vector.tensor_mul(ot_sb[p][0:64, qsl], our[0][0:64, :],
                                     bcps[0:64, :])
                nc.vector.tensor_mul(ot_sb[p][64:128, qsl], our[1][0:64, :],
                                     bcps[64:128, :])

            def emit_outproj(f, qc, ot_sb):
                for st in range(qc * (NSTF // NQC), (qc + 1) * (NSTF // NQC)):
                    osb = outsb_pool.tile([128, D], dt.float32, tag="osb",
                                          name=f"osb_{f}_{st}")
                    for nch in range(2):
                        n0, n1 = (0, 512) if nch == 0 else (512, D)
                        ps = mm_ps.tile([128, 512], dt.float32, tag="mm",
                                        name=f"ops_{f}_{st}_{nch}")
                        for d in range(NDC):
                            nc.tensor.matmul(
                                ps[:, : n1 - n0],
                                ot_sb[d][:, st * 128 : (st + 1) * 128],
                                wo_t[d][:, n0:n1],
                                start=(d == 0),
                                stop=(d == NDC - 1),
                            )
                        nc.vector.tensor_add(osb[:, n0:n1], ps[:, : n1 - n0],
                                             bias_t[:, n0:n1])
                    nc.sync.dma_start(
                        out[f * S + st * 128 : f * S + (st + 1) * 128, :], osb[:])

            # ================= emission order =================
            # Phase-separated like the proven baseline: dense per-frame
            # projection burst (v+qk interleaved per chunk), rope, then the
            # attention block; out-proj emitted after the frame's attention so
            # it overlaps the next frame's projection burst. qc-major attention
            # lets outproj(qc0) fill attention(qc1) stalls.
            xts_f = {}
            frames = {}
            xts_f[0] = dma_x(0)
            dma_late_consts()
            frames[0] = alloc_frame(0)
            for f in range(FPC):
                qk_q, qk_k, v_sb, ot_sb = frames[f]
                xts = xts_f[f]
                # projection burst: qk pairs first (their weights land first)
                for p in range(NPAIR):
                    emit_qk_pair(f, p, xts, qk_q, qk_k)
                for lst in range(NSTF):
                    emit_v_stile(f, xts, v_sb, lst)
                # attention, qc-major
                for qc in range(NQC):
                    for p in range(NPAIR):
                        emit_attn_unit(f, qc, p, qk_q, qk_k, v_sb, ot_sb)
                # next frame's inputs + tiles before out-proj so its DMA runs early
                if f + 1 < FPC:
                    xts_f[f + 1] = dma_x(f + 1)
                    frames[f + 1] = alloc_frame(f + 1)
                emit_outproj(f, 0, ot_sb)
                emit_outproj(f, 1, ot_sb)

    nc.compile()
    return nc


def _prep_inputs(x, w_qkv, w_out, b_out, H, W):
    """Host-side prep: shard + transpose + cast. Returns per-core in_maps."""
    S = H * W
    SL = FPC * S
    nframes = x.shape[0] * x.shape[1]
    ncores = nframes // FPC
    xf = np.asarray(x, dtype=np.float32).reshape(nframes, S, D)

    wqkvT = np.ascontiguousarray(np.asarray(w_qkv, np.float32).T).astype(BF16)
    wqkT = np.ascontiguousarray(wqkvT[:, : 2 * D])
    wvT = np.ascontiguousarray(wqkvT[:, 2 * D :])
    w_outT = np.ascontiguousarray(np.asarray(w_out, np.float32).T).astype(BF16)
    cos, sin = _host_rope(H, W, HD)  # [S, 64]
    cosP = np.tile(cos.T, (2, 1)).astype(BF16)  # [128, S]
    sinP = np.tile(sin.T, (2, 1)).astype(BF16)
    rotT = _rot_matT().astype(BF16)
    ones64 = np.ones((1, 64), dtype=BF16)
    bias_rep = np.tile(np.asarray(b_out, np.float32)[None, :], (128, 1)).astype(BF16)

    in_maps = []
    for c in range(ncores):
        shard = xf[c * FPC : (c + 1) * FPC].reshape(SL, D)
        xT = np.ascontiguousarray(shard.T).astype(BF16)  # [768, SL]
        in_maps.append(
            dict(
                xT=xT,
                wqkT=wqkT,
                wvT=wvT,
                w_outT=w_outT,
                cosP=cosP,
                sinP=sinP,
                rotT=rotT,
                ones64=ones64,
                bias_rep=bias_rep,
            )
        )
    return in_maps


def run(x, w_qkv, w_out, b_out, trace=False):
    from concourse import bass_utils

    Hd, Wd = x.shape[2], x.shape[3]
    key = (Hd, Wd)
    if key not in _built:
        _built[key] = build_nc(Hd, Wd)
    nc = _built[key]
    in_maps = _prep_inputs(x, w_qkv, w_out, b_out, Hd, Wd)
    res = bass_utils.run_bass_kernel_spmd(
        nc, in_maps, core_ids=list(range(len(in_maps))), trace=trace
    )
    outs = [r["out"] for r in res.results]
    full = np.concatenate(outs, axis=0).reshape(B, T, Hd, Wd, D).astype(np.float32)
    return full, res


def kernel(x, w_qkv, w_out, b_out):
    full, _ = run(x, w_qkv, w_out, b_out, trace=False)
    return full
